# revision 33
# baseline (speedup 1.0000x reference)
"""Multi-head attention (B=2, S=2048, D=768, H=12, Dh=64) on 8 TRN2 cores.

Sharding: core = (batch b = core//4, head-group g = core%4 of 3 heads).
Each core computes its 3 heads' attention for its batch and a partial
output projection [S, 768]; host sums the 4 group-partials per batch and
adds b_proj.

v2 design (fully software-pipelined, ACT-exp is the pacing engine):
  - All matmul operands f16 (storage); accumulation fp32 in PSUM.  f16
    rounding (2^-11) on q/k contributes ~2e-4 abs error on the /8-scaled
    scores -- negligible vs the 2e-3 budget.
  - x arrives as 4 per-qt-slice tiles [128, 6, 512] (host pre-packed,
    partition-major so each DMA is 128 x 6KB contiguous descriptors).
    As slice s lands: K-stream matmuls for that slice, then q for qt=s.
    Scores for qt0 (and the first exp) start at ~13us instead of ~70us.
  - Scores: ST[k,q] via row-tiled K=64 pairs (heads 0,1 packed in one
    [128,S] tile; head 2 duplicated into both halves).  Rounds of 1
    chunk (p01) / 2 chunks (h2): each round = 2 dual-issued MMs into a
    [128,2,512] PSUM region (ring of 2) consumed by ONE exp (FD=1024).
  - exp on ACT with scale=1/8 folded; masks in the reference are scaled
    by +1e-9 (numerically zero in fp32) and are elided.  No
    max-subtraction needed (|scores/8| < ~3).
  - Row-sums come free via a ones-column appended to V (context matmul
    output row 64 = softmax denominator Z).
  - Normalize: reciprocal_approx_fast (DVE custom op, ~5x faster than
    reciprocal) on Z, partition_broadcast on GPSIMD (no DRAM round-trip),
    one TT multiply straight out of the ct PSUM bank.
  - Projection per qt right after its 3 normalizes, interleaved with the
    next qt's score rounds; output tiles DMA out as they finish.
  - Scalar queue carries ONLY activations; loads on sync (HWDGE),
    biases/ones/out on gpsimd (SWDGE).
"""

import numpy as np

B = 2
S = 2048
D = 768
NH = 12
DH = 64
NCORES = 8
P = 128
KCH = D // P          # 6 contraction chunks for the QKV projection
NQT = S // 512        # 4 query tiles of 512
NKC = S // P          # 16 key chunks of 128

_CACHE = {}


def _build():
    import concourse.mybir as mybir
    import concourse.tile as tile
    from concourse import bacc

    F32 = mybir.dt.float32
    F16 = mybir.dt.float16
    EXP = mybir.ActivationFunctionType.Exp

    nc = bacc.Bacc(target_bir_lowering=False, debug=False)

    xtq_d = nc.dram_tensor("xtq", [NQT, P, KCH, 512], F16, kind="ExternalInput")
    wq01_d = nc.dram_tensor("wq01", [P, KCH, P], F16, kind="ExternalInput")
    wq2d_d = nc.dram_tensor("wq2d", [P, KCH, P], F16, kind="ExternalInput")
    wk01_d = nc.dram_tensor("wk01", [P, KCH, P], F16, kind="ExternalInput")
    wk2d_d = nc.dram_tensor("wk2d", [P, KCH, P], F16, kind="ExternalInput")
    wv_d = nc.dram_tensor("wv", [P, KCH, 3 * DH], F16, kind="ExternalInput")
    wp_d = nc.dram_tensor("wp", [3 * DH, D], F16, kind="ExternalInput")
    bqk_d = nc.dram_tensor("bqk", [P, 4], F32, kind="ExternalInput")
    bv_d = nc.dram_tensor("bv", [1, 3 * DH], F32, kind="ExternalInput")
    out_d = nc.dram_tensor("out", [S, D], F32, kind="ExternalOutput")

    with tile.TileContext(nc) as tc:
        with (
            tc.sbuf_pool(name="pw", bufs=1) as pw,
            tc.sbuf_pool(name="pqk", bufs=1) as pqk,
            tc.sbuf_pool(name="pv", bufs=1) as pv,
            tc.sbuf_pool(name="pctn", bufs=1) as pctn,
            tc.sbuf_pool(name="ppt", bufs=1) as ppt,
            tc.sbuf_pool(name="pz", bufs=1) as pz,
            tc.sbuf_pool(name="pout", bufs=1) as pout,
            tc.psum_pool(name="pR", bufs=1) as pR,
        ):
            # ---- weights / biases ----
            wq01 = pw.tile([P, KCH, P], F16)
            wq2d = pw.tile([P, KCH, P], F16)
            wk01 = pw.tile([P, KCH, P], F16)
            wk2d = pw.tile([P, KCH, P], F16)
            wv = pw.tile([P, KCH, 3 * DH], F16)
            wp01 = pw.tile([P, D], F16)
            wp2 = pw.tile([DH, D], F16)
            bqk = pw.tile([P, 4], F32)      # bq01 | bq2d | bk01 | bk2d
            bv1 = pw.tile([1, 3 * DH], F32)
            bvb = pw.tile([P, 3 * DH], F32)

            # Weights go on the scalar HWDGE ring (done well before the
            # first exp needs the queue); sync is reserved for x tiles.
            nc.scalar.dma_start(out=wk01, in_=wk01_d.ap())
            nc.scalar.dma_start(out=wq01, in_=wq01_d.ap())
            nc.scalar.dma_start(out=wk2d, in_=wk2d_d.ap())
            nc.scalar.dma_start(out=wq2d, in_=wq2d_d.ap())
            nc.scalar.dma_start(out=wv, in_=wv_d.ap())

            # gpsimd (SWDGE) carries the small ones.  No to_broadcast
            # DMAs: a [1,n] load + gpsimd partition_broadcast avoids the
            # thousands of tiny DMA packets that starve the HW rings.
            nc.gpsimd.dma_start(out=bqk, in_=bqk_d.ap())
            nc.gpsimd.dma_start(out=bv1, in_=bv_d.ap())
            nc.gpsimd.partition_broadcast(bvb, bv1, channels=P)

            # ---- persistent activations ----
            q01 = pqk.tile([P, S], F16)
            q2d = pqk.tile([P, S], F16)
            k01 = pqk.tile([P, S], F16)
            k2d = pqk.tile([P, S], F16)
            v3 = pv.tile([P, NKC, 3, DH + 1], F16)
            ctn01 = pctn.tile([P, NQT, 512], F16)
            ctn2 = pctn.tile([DH, NQT, 512], F16)

            # ones-column for the free softmax denominator (row 64 of each
            # context tile) -- DVE memset, not a tiny-packet broadcast DMA.
            nc.vector.memset(v3[:, :, :, DH:DH + 1], 1.0)

            with tc.sbuf_pool(name="px", bufs=1) as px, \
                 tc.psum_pool(name="pload", bufs=1) as pload:
                xts = []
                for qs in range(NQT):
                    xt = px.tile([P, KCH, 512], F16, name=f"xts{qs}")
                    xts.append(xt)
                for qs in range(NQT):
                    nc.sync.dma_start(out=xts[qs], in_=xtq_d.ap()[qs])
                nc.gpsimd.dma_start(out=wp01, in_=wp_d.ap()[0:P, :])
                nc.gpsimd.dma_start(out=wp2, in_=wp_d.ap()[P:P + DH, :])

                # ---- QKV streams ----
                # Order: all k01 chains first, then q01/q2d for qt0, so the
                # first score round unblocks after ~6 chains, not 16.
                def chain(dst, w, bias_i, qs):
                    acc = pload.tile([P, 512], F32, tag="acc", bufs=2,
                                     name=f"acc{qs}", uniquify=True)
                    for c in range(KCH):
                        nc.tensor.matmul(
                            acc, w[:, c, :], xts[qs][:, c, :],
                            start=(c == 0), stop=(c == KCH - 1))
                    nc.vector.tensor_scalar_add(
                        out=dst[:, qs * 512:(qs + 1) * 512], in0=acc,
                        scalar1=bqk[:, bias_i:bias_i + 1])

                # ---- score rounds + exp ----
                pt = {}

                def get_pt(qt):
                    # pt ring: 2 qt deep
                    pt01 = ppt.tile([P, NKC, 2, 512], F16, tag="pt01", bufs=2,
                                    name=f"pt01_{qt}", uniquify=True)
                    pt2 = ppt.tile([P, NKC, 512], F16, tag="pt2", bufs=2,
                                   name=f"pt2_{qt}", uniquify=True)
                    pt[qt] = (pt01, pt2)

                def rounds_p01(qt):
                    pt01 = pt[qt][0]
                    qsl = slice(qt * 512, (qt + 1) * 512)
                    for c in range(NKC):
                        reg = pR.tile([P, 2, 512], F32, tag="sc", bufs=2,
                                      name=f"r{qt}_{c}", uniquify=True)
                        nc.tensor.matmul(
                            reg[:, 0, :], k01[0:DH, c * P:(c + 1) * P],
                            q01[0:DH, qsl], start=True, stop=True)
                        nc.tensor.matmul(
                            reg[:, 1, :], k01[DH:P, c * P:(c + 1) * P],
                            q01[DH:P, qsl], start=True, stop=True)
                        nc.scalar.activation(pt01[:, c, :, :], reg, EXP,
                                             scale=0.125)

                def rounds_h2(qt):
                    pt2 = pt[qt][1]
                    qsl = slice(qt * 512, (qt + 1) * 512)
                    for j in range(NKC // 2):
                        reg = pR.tile([P, 2, 512], F32, tag="sc", bufs=2,
                                      name=f"r2{qt}_{j}", uniquify=True)
                        nc.tensor.matmul(
                            reg[:, 0, :], k2d[0:DH, (2 * j) * P:(2 * j + 1) * P],
                            q2d[0:DH, qsl], start=True, stop=True)
                        nc.tensor.matmul(
                            reg[:, 1, :],
                            k2d[DH:P, (2 * j + 1) * P:(2 * j + 2) * P],
                            q2d[DH:P, qsl], start=True, stop=True)
                        nc.scalar.activation(pt2[:, 2 * j:2 * j + 2, :], reg,
                                             EXP, scale=0.125)

                # emission order = scheduler priority: q01[0] right after
                # k01[0] so its strict-FIFO DVE evac isn't stuck behind
                # chains gated on later x slices; all q chains before V so
                # qt2/qt3 rounds never starve.
                chain(k01, wk01, 2, 0)
                chain(q01, wq01, 0, 0)
                get_pt(0)
                rounds_p01(0)
                for qs in (1, 2, 3):
                    chain(k01, wk01, 2, qs)
                chain(k2d, wk2d, 3, 0)
                chain(q2d, wq2d, 1, 0)
                rounds_h2(0)
                for qs in (1, 2, 3):
                    chain(k2d, wk2d, 3, qs)
                chain(q01, wq01, 0, 1)
                chain(q2d, wq2d, 1, 1)
                get_pt(1)
                rounds_p01(1)
                rounds_h2(1)
                for qs in (2, 3):
                    chain(q01, wq01, 0, qs)
                    chain(q2d, wq2d, 1, qs)

                # ---- V matmuls (fill PE idle while qt0/qt1 exps run) ----
                for sc in range(NKC):
                    vacc = pload.tile([P, 3 * DH], F32, tag="vacc", bufs=2,
                                      name=f"vacc{sc}", uniquify=True)
                    qs, i = divmod(sc, 4)
                    for c in range(KCH):
                        nc.tensor.matmul(
                            vacc, xts[qs][:, c, i * P:(i + 1) * P], wv[:, c, :],
                            start=(c == 0), stop=(c == KCH - 1))
                    for h in range(3):
                        nc.vector.tensor_add(
                            v3[:, sc, h, 0:DH],
                            vacc[:, h * DH:(h + 1) * DH],
                            bvb[:, h * DH:(h + 1) * DH])

            # pload (4 banks) closed; ct + proj psum pools take its place.
            with tc.psum_pool(name="pct", bufs=1) as pct, \
                 tc.psum_pool(name="pproj", bufs=1) as pproj:

                def contexts(qt, heads=(0, 1, 2)):
                    pt01, pt2 = pt[qt]
                    for h in heads:
                        ct = pct.tile([DH + 1, 512], F32, tag="ct", bufs=2,
                                      name=f"ct{h}_{qt}", uniquify=True)
                        for c in range(NKC):
                            rhs = pt01[:, c, h, :] if h < 2 else pt2[:, c, :]
                            nc.tensor.matmul(ct, v3[:, c, h, :], rhs,
                                             start=(c == 0), stop=(c == NKC - 1))
                        # normalize.  Evacuate PSUM immediately (frees the
                        # ct slot for the next head's accumulation), with
                        # the Z row hopped to partition 0 (cross-partition
                        # tensor_copy is legal; the custom-DVE recip and
                        # the gpsimd broadcast ucode need base-0 APs).
                        ctu = pz.tile([DH, 512], F32, tag="ctu", bufs=2,
                                      name=f"cu{h}{qt}", uniquify=True)
                        nc.vector.tensor_copy(ctu, ct[0:DH, :])
                        z0 = pz.tile([1, 512], F32, tag="z0", bufs=2,
                                     name=f"z0{h}{qt}", uniquify=True)
                        nc.vector.tensor_copy(z0, ct[DH:DH + 1, :])
                        rz = pz.tile([1, 512], F32, tag="rz", bufs=2,
                                     name=f"rz{h}{qt}", uniquify=True)
                        nc.vector.reciprocal_approx_fast(out=rz, in_=z0)
                        rp = pz.tile([DH, 512], F32, tag="rp", bufs=2,
                                     name=f"rp{h}{qt}", uniquify=True)
                        nc.gpsimd.partition_broadcast(rp, rz, channels=DH)
                        if h == 0:
                            dst = ctn01[0:DH, qt, :]
                        elif h == 1:
                            dst = ctn01[DH:P, qt, :]
                        else:
                            dst = ctn2[:, qt, :]
                        nc.vector.tensor_mul(dst, ctu, rp)

                def proj(qt, evac):
                    for st in range(4):
                        sl = slice(st * P, (st + 1) * P)
                        ppA = pproj.tile([P, 512], F32, tag="ppA", bufs=1,
                                         name=f"ppA{qt}{st}", uniquify=True)
                        ppB = pproj.tile([P, 256], F32, tag="ppB", bufs=1,
                                         name=f"ppB{qt}{st}", uniquify=True)
                        nc.tensor.matmul(ppA, ctn01[:, qt, sl],
                                         wp01[:, 0:512], start=True, stop=False)
                        nc.tensor.matmul(ppA, ctn2[:, qt, sl],
                                         wp2[:, 0:512], start=False, stop=True)
                        nc.tensor.matmul(ppB, ctn01[:, qt, sl],
                                         wp01[:, 512:D], start=True, stop=False)
                        nc.tensor.matmul(ppB, ctn2[:, qt, sl],
                                         wp2[:, 512:D], start=False, stop=True)
                        stage = pout.tile([P, D], F32, tag="stage", bufs=3,
                                          name=f"st{qt}{st}", uniquify=True)
                        if evac == "scalar":
                            # tail: ACT and DVE are both idle -- split the
                            # two evacuations across them so the ppA/ppB
                            # WAR chains advance in parallel
                            nc.scalar.copy(stage[:, 0:512], ppA)
                            nc.vector.tensor_copy(stage[:, 512:D], ppB)
                        else:
                            nc.vector.tensor_copy(stage[:, 0:512], ppA)
                            nc.vector.tensor_copy(stage[:, 512:D], ppB)
                        r0 = qt * 512 + st * P
                        nc.gpsimd.dma_start(out=out_d.ap()[r0:r0 + P, :], in_=stage)

                contexts(0)
                get_pt(2)
                rounds_p01(2)
                rounds_h2(2)
                proj(0, "vector")
                contexts(1)
                get_pt(3)
                rounds_h2(3)
                rounds_p01(3)
                proj(1, "vector")
                contexts(2)
                contexts(3, heads=(2,))
                proj(2, "vector")
                contexts(3, heads=(0, 1))
                proj(3, "scalar")

    nc.compile()
    return nc


def _get_nc():
    if "nc" not in _CACHE:
        _CACHE["nc"] = _build()
    return _CACHE["nc"]


def kernel(x, attention_mask, w_qkv, b_qkv, w_proj, b_proj, _trace=False):
    from concourse.bass_utils import run_bass_kernel_spmd

    x = np.asarray(x, dtype=np.float32)
    w_qkv = np.asarray(w_qkv, dtype=np.float32)
    b_qkv = np.asarray(b_qkv, dtype=np.float32)
    w_proj = np.asarray(w_proj, dtype=np.float32)
    b_proj = np.asarray(b_proj, dtype=np.float32)

    def wtile(cols):
        # [768, m] -> [128, 6, m] f16 (partition-major chunk layout)
        m = cols.shape[1]
        return np.ascontiguousarray(
            cols.reshape(KCH, P, m).transpose(1, 0, 2).astype(np.float16))

    in_maps = []
    for core in range(NCORES):
        b, g = divmod(core, 4)
        base = g * 3 * DH
        wq2 = w_qkv[:, base + 2 * DH:base + 3 * DH]
        wk2 = w_qkv[:, D + base + 2 * DH:D + base + 3 * DH]
        bq2 = b_qkv[base + 2 * DH:base + 3 * DH]
        bk2 = b_qkv[D + base + 2 * DH:D + base + 3 * DH]
        xtq = np.ascontiguousarray(
            x[b].reshape(NQT, 512, KCH, P).transpose(0, 3, 2, 1).astype(np.float16))
        in_maps.append({
            "xtq": xtq,
            "wq01": wtile(w_qkv[:, base:base + 2 * DH]),
            "wq2d": wtile(np.concatenate([wq2, wq2], axis=1)),
            "wk01": wtile(w_qkv[:, D + base:D + base + 2 * DH]),
            "wk2d": wtile(np.concatenate([wk2, wk2], axis=1)),
            "wv": wtile(w_qkv[:, 2 * D + base:2 * D + base + 3 * DH]),
            "wp": np.ascontiguousarray(
                w_proj[base:base + 3 * DH, :].astype(np.float16)),
            "bqk": np.ascontiguousarray(np.stack([
                b_qkv[base:base + 2 * DH],
                np.concatenate([bq2, bq2]),
                b_qkv[D + base:D + base + 2 * DH],
                np.concatenate([bk2, bk2]),
            ], axis=1).astype(np.float32)),
            "bv": np.ascontiguousarray(
                b_qkv[2 * D + base:2 * D + base + 3 * DH].reshape(1, 3 * DH)),
        })

    nc = _get_nc()
    # Warmup execution: the very first run after NEFF load can race the
    # ACT function-table load, corrupting a few exp results. Tables are
    # resident afterwards, so the second run is clean — return that one.
    run_bass_kernel_spmd(nc, in_maps, list(range(NCORES)), trace=False)
    res = run_bass_kernel_spmd(nc, in_maps, list(range(NCORES)), trace=_trace)
    if _trace:
        _CACHE["last_result"] = res

    out = np.zeros((B, S, D), dtype=np.float32)
    for core in range(NCORES):
        b = core // 4
        out[b] += res.results[core]["out"]
    out += b_proj[None, None, :]
    return out
